# revision 13
# baseline (speedup 1.0000x reference)
"""Cross-Scale Non-Local Attention kernel for 8x Trainium2 NeuronCores.

Data-parallel over batch: each of the 8 cores processes one sample
(B=8, H=W=64, C=64). Per-core Bass/Tile program:

  1. x loaded in 4 chunks; each chunk is PE-transposed to channel-major
     xT [c=64, 4096] with g/theta matmuls interleaved per chunk so the
     tensor engine stays dense (HAM stays un-throttled).
  2. g [pix, 64] = prelu(xT.T @ g_w)/6, bounced to DRAM as g_pad
     [72,72,64] (zero borders = conv-transpose SAME padding); the 18
     shifted dynamic-filter views kg[q,qw,kb] [n=128, (r rw c)=1024] are
     gathered back by strided DMA.
  3. thetaT = prelu(theta_w.T @ xT) into a zero-padded 66x66 buffer.
  4. phi from 4-tap bilinear downsample; 3x3 patches + L2 norms in the
     padded [18,18] domain; s10[n] = 10/max(norm,1e-6).
  5. Per pixel-chunk ch: scoresT [n, pix] via 9 shifted-window matmuls
     (K=32); E = exp(s10[n]*score) (no max subtraction - |10*score| < 80
     so fp32 exp cannot overflow); S[pix] = ones.T @ E via matmul;
     rb = 1/S broadcast to 128 partitions.
  6. Deconv as polyphase conv-transpose with swapped operands,
     interleaved one chunk behind the scores loop:
     psum[rc 128, pix 512] += kg[q,qw,kb][:, u*128:(u+1)*128].T @ E_window
     over 18 shifts; the softmax divide is fused into the PSUM drain
     (tr_in = psum * rb); PE-transpose back to [pix, rc]; transposes are
     software-pipelined one tile behind the matmuls so PE never waits on
     the DVE drain.

All matmuls use float32r (FP22 multiply, FP32 accumulate, full PE rate).
"""

import numpy as np

_CACHE = {}

# Problem constants (hardcoded per harness contract)
B = 8
H = W = 64
C = 64
CI = 32
HS = WS = 16
N = 256          # HS*WS low-res positions
PH = 66          # padded attn/theta spatial extent (64 + 1 halo each side)
GP = 72          # padded g spatial extent (64 + 4 each side)


def _build_nc():
    import concourse.bass as bass
    import concourse.tile as tile
    from concourse import bacc, mybir
    from concourse.masks import make_identity
    from contextlib import ExitStack

    F32 = mybir.dt.float32
    F32R = mybir.dt.float32r
    Alu = mybir.AluOpType
    Act = mybir.ActivationFunctionType

    def r_(ap):
        return ap.bitcast(F32R)

    nc = bacc.Bacc("TRN2", debug=False)

    x_h = nc.dram_tensor("x", [H, W, C], F32, kind="ExternalInput")
    thw_h = nc.dram_tensor("theta_w", [C, CI], F32, kind="ExternalInput")
    thb_h = nc.dram_tensor("theta_b", [CI], F32, kind="ExternalInput")
    tha_h = nc.dram_tensor("theta_alpha", [CI], F32, kind="ExternalInput")
    phw_h = nc.dram_tensor("phi_w", [C, CI], F32, kind="ExternalInput")
    phb_h = nc.dram_tensor("phi_b", [CI], F32, kind="ExternalInput")
    pha_h = nc.dram_tensor("phi_alpha", [CI], F32, kind="ExternalInput")
    gw_h = nc.dram_tensor("g_w", [C, C], F32, kind="ExternalInput")
    gb_h = nc.dram_tensor("g_b", [C], F32, kind="ExternalInput")
    ga_h = nc.dram_tensor("g_alpha", [C], F32, kind="ExternalInput")
    y_h = nc.dram_tensor("y", [4 * H, 4 * W, C], F32, kind="ExternalOutput")
    import os
    kdebug = bool(os.environ.get("KDEBUG"))
    if kdebug:
        dbgE_h = nc.dram_tensor("dbgE", [128, 2, PH, PH], F32,
                                kind="ExternalOutput")
        dbgS_h = nc.dram_tensor("dbgS", [8, 512], F32, kind="ExternalOutput")
        dbgT_h = nc.dram_tensor("dbgT", [8, 128, 512], F32,
                                kind="ExternalOutput")
        dbgP_h = nc.dram_tensor("dbgP", [8, 128, 512], F32,
                                kind="ExternalOutput")
        dbgR_h = nc.dram_tensor("dbgR", [8, 128, 512], F32,
                                kind="ExternalOutput")

    with tile.TileContext(nc) as tc, ExitStack() as top:
        ec = top.enter_context

        consts = ec(tc.tile_pool(name="consts", bufs=1))
        persist = ec(tc.tile_pool(name="persist", bufs=1))
        phip = ec(tc.tile_pool(name="phip", bufs=1))
        dramp = ec(tc.tile_pool(name="dramp", bufs=1, space="DRAM"))
        staging = ec(tc.tile_pool(name="staging", bufs=3))
        ps_misc = ec(tc.tile_pool(name="ps_misc", bufs=2, space="PSUM"))
        ps_sc = ec(tc.tile_pool(name="ps_sc", bufs=2, space="PSUM"))
        ps_d = ec(tc.tile_pool(name="ps_d", bufs=3, space="PSUM"))
        ps_tr = ec(tc.tile_pool(name="ps_tr", bufs=1, space="PSUM"))

        # ---- constants / weights in SBUF ----
        ident = consts.tile([128, 128], F32)
        make_identity(nc, ident)
        thw_sb = consts.tile([C, CI], F32)
        nc.sync.dma_start(out=r_(thw_sb), in_=r_(thw_h.ap()))
        phw_sb = consts.tile([C, CI], F32)
        nc.sync.dma_start(out=r_(phw_sb), in_=r_(phw_h.ap()))
        gw_sb = consts.tile([C, C], F32)
        nc.sync.dma_start(out=r_(gw_sb), in_=r_(gw_h.ap()))
        thb_sb = consts.tile([CI, 1], F32)
        nc.sync.dma_start(out=thb_sb, in_=thb_h.ap().unsqueeze(1))
        tha_sb = consts.tile([CI, 1], F32)
        nc.sync.dma_start(out=tha_sb, in_=tha_h.ap().unsqueeze(1))
        phb_sb = consts.tile([CI, 1], F32)
        nc.sync.dma_start(out=phb_sb, in_=phb_h.ap().unsqueeze(1))
        pha_sb = consts.tile([CI, 1], F32)
        nc.sync.dma_start(out=pha_sb, in_=pha_h.ap().unsqueeze(1))
        gb_row = consts.tile([1, C], F32)
        nc.sync.dma_start(out=gb_row, in_=gb_h.ap().unsqueeze(0))
        ga_row = consts.tile([1, C], F32)
        nc.sync.dma_start(out=ga_row, in_=ga_h.ap().unsqueeze(0))
        gb_bc = consts.tile([128, C], F32)
        nc.gpsimd.partition_broadcast(gb_bc, gb_row)
        ga6_bc = consts.tile([128, C], F32)
        nc.gpsimd.partition_broadcast(ga6_bc, ga_row)
        nc.vector.tensor_scalar_mul(ga6_bc, ga6_bc, 1.0 / 6.0)
        z66 = consts.tile([128, PH], F32)
        nc.vector.memset(z66, 0.0)
        o1 = consts.tile([128, 1], F32)
        nc.vector.memset(o1, 1.0)
        ones32 = consts.tile([CI, 1], F32)
        nc.vector.tensor_copy(out=r_(ones32), in_=o1[:CI])
        ones128 = consts.tile([128, 1], F32)
        nc.vector.tensor_copy(out=r_(ones128), in_=o1)
        s10T = consts.tile([128, 2], F32)

        # ---- persistent activation buffers ----
        thetaT_pad = persist.tile([CI, PH, PH], F32)
        nc.vector.tensor_copy(out=r_(thetaT_pad[:, 0, :]), in_=z66[:CI])
        nc.vector.tensor_copy(out=r_(thetaT_pad[:, PH - 1, :]), in_=z66[:CI])
        nc.vector.tensor_copy(out=r_(thetaT_pad[:, :, 0]), in_=z66[:CI])
        nc.vector.tensor_copy(out=r_(thetaT_pad[:, :, PH - 1]), in_=z66[:CI])
        attnT = persist.tile([128, 2, PH, PH], F32)
        for kb in range(2):
            nc.vector.tensor_copy(out=r_(attnT[:, kb, 0, :]), in_=z66)
            nc.vector.tensor_copy(out=r_(attnT[:, kb, PH - 1, :]), in_=z66)
            nc.vector.tensor_copy(out=r_(attnT[:, kb, :, 0]), in_=z66)
            nc.vector.tensor_copy(out=r_(attnT[:, kb, :, PH - 1]), in_=z66)
        phi_patchT = persist.tile([CI, 3, 3, N], F32)

        phiT_pad = phip.tile([CI, 18, 18], F32)
        nc.vector.memset(phiT_pad, 0.0)
        n2p = phip.tile([1, 324], F32)
        nrm = phip.tile([1, N], F32)
        phi_inT = phip.tile([C, HS, WS], F32)

        g_pad = dramp.tile([GP, GP, C], F32)
        sbounce = dramp.tile([N], F32)

        # zero-fill g_pad from a zeroed staging tile (5x64K + 8x512 tail)
        zt = staging.tile([128, 512], F32, tag="stg")
        nc.vector.memset(zt, 0.0)
        gpf = g_pad.rearrange("h w c -> (h w c)")
        head = gpf[: 5 * 65536].rearrange("(k p f) -> k p f", p=128, f=512)
        for k in range(5):
            nc.sync.dma_start(out=head[k], in_=zt)
        tail = gpf[5 * 65536:].rearrange("(p f) -> p f", f=512)
        nc.sync.dma_start(out=tail, in_=zt[: tail.shape[0], :])

        with ExitStack() as st1:
            e1 = st1.enter_context
            xp_pool = e1(tc.tile_pool(name="xp_pool", bufs=1))
            xt_pool = e1(tc.tile_pool(name="xt_pool", bufs=1))
            gsb_pool = e1(tc.tile_pool(name="gsb_pool", bufs=1))
            ttmp = e1(tc.tile_pool(name="ttmp", bufs=2))
            gtmp = e1(tc.tile_pool(name="gtmp", bufs=3))

            xP = xp_pool.tile([128, 32, C], F32)
            x_r = x_h.ap().rearrange("h w c -> (h w) c").rearrange(
                "(t p) c -> p t c", p=128)
            xT = xt_pool.tile([C, H, W], F32)
            xTf = xT.rearrange("c h w -> c (h w)")
            g_sb = gsb_pool.tile([128, 32, C], F32)

            def theta_chunk(ch):
                h0 = ch * 8
                ps_t = ps_misc.tile([CI, 512], F32, tag="m", name=f"ps_t{ch}")
                nc.tensor.matmul(
                    ps_t, r_(thw_sb), r_(xTf[:, ch * 512:(ch + 1) * 512]),
                    start=True, stop=True)
                t_lin = ttmp.tile([CI, 8, W], F32, tag="tl")
                nc.vector.tensor_scalar_add(
                    t_lin.rearrange("p a b -> p (a b)"), ps_t, thb_sb)
                t_neg = ttmp.tile([CI, 8, W], F32, tag="tn")
                nc.vector.tensor_scalar(
                    t_neg.rearrange("p a b -> p (a b)"),
                    t_lin.rearrange("p a b -> p (a b)"),
                    0.0, tha_sb, Alu.min, Alu.mult)
                nc.vector.scalar_tensor_tensor(
                    out=r_(thetaT_pad[:, 1 + h0:9 + h0, 1:65]),
                    in0=t_lin, scalar=0.0, in1=t_neg,
                    op0=Alu.max, op1=Alu.add)

            # interleaved: x chunk DMA -> transposes -> g matmuls -> theta
            for xc in range(4):
                nc.sync.dma_start(
                    out=xP[:, xc * 8:(xc + 1) * 8, :],
                    in_=x_r[:, xc * 8:(xc + 1) * 8, :])
                for t in range(xc * 8, (xc + 1) * 8):
                    ps_x = ps_misc.tile([C, 128], F32, tag="m",
                                        name=f"ps_x{t}")
                    nc.tensor.transpose(ps_x, xP[:, t, :], ident)
                    nc.scalar.copy(
                        out=r_(xTf[:, t * 128:(t + 1) * 128]), in_=ps_x)
                for t in range(xc * 8, (xc + 1) * 8):
                    ps_g = ps_misc.tile([128, C], F32, tag="m",
                                        name=f"ps_g{t}")
                    nc.tensor.matmul(
                        ps_g, r_(xTf[:, t * 128:(t + 1) * 128]), r_(gw_sb),
                        start=True, stop=True)
                    gv = gtmp.tile([128, C], F32, tag="gv")
                    nc.vector.tensor_add(gv, ps_g, gb_bc)
                    gm1 = gtmp.tile([128, C], F32, tag="gm1")
                    nc.vector.tensor_scalar_max(gm1, gv, 0.0)
                    nc.vector.tensor_scalar_min(gv, gv, 0.0)
                    nc.vector.tensor_mul(gv, gv, ga6_bc)
                    nc.vector.scalar_tensor_tensor(
                        out=g_sb[:, t, :], in0=gm1, scalar=1.0 / 6.0, in1=gv,
                        op0=Alu.mult, op1=Alu.add)
                theta_chunk(2 * xc)
                theta_chunk(2 * xc + 1)

            gint = g_pad[4:68, 4:68, :].rearrange("(t a) w c -> a w t c", a=2)
            for p1 in range(2):
                nc.sync.dma_start(
                    out=gint[p1], in_=g_sb[p1 * 64:(p1 + 1) * 64, :, :])

            # phi: bilinear downsample (4-tap avg) then 1x1 conv + prelu
            xv = xT.rearrange("c (hq hs) (wq ws) -> c hq hs wq ws", hs=4, ws=4)
            nc.vector.tensor_add(r_(phi_inT), xv[:, :, 1, :, 1],
                                 xv[:, :, 1, :, 2])
            nc.vector.tensor_add(r_(phi_inT), phi_inT, xv[:, :, 2, :, 1])
            nc.vector.tensor_add(r_(phi_inT), phi_inT, xv[:, :, 2, :, 2])
            nc.vector.tensor_scalar_mul(r_(phi_inT), phi_inT, 0.25)
            ps_phi = ps_misc.tile([CI, N], F32, tag="m")
            nc.tensor.matmul(
                ps_phi, r_(phw_sb), r_(phi_inT.rearrange("c a b -> c (a b)")),
                start=True, stop=True)
            p_lin = ttmp.tile([CI, HS, WS], F32, tag="pl")
            nc.vector.tensor_scalar_add(
                p_lin.rearrange("p a b -> p (a b)"), ps_phi, phb_sb)
            p_neg = ttmp.tile([CI, HS, WS], F32, tag="pn")
            nc.vector.tensor_scalar(
                p_neg.rearrange("p a b -> p (a b)"),
                p_lin.rearrange("p a b -> p (a b)"),
                0.0, pha_sb, Alu.min, Alu.mult)
            nc.vector.scalar_tensor_tensor(
                out=phiT_pad[:, 1:17, 1:17],
                in0=p_lin, scalar=0.0, in1=p_neg,
                op0=Alu.max, op1=Alu.add)

            # phi patches (padded windows) + per-patch L2 norm -> s10
            for kh in range(3):
                for kw in range(3):
                    nc.vector.tensor_copy(
                        out=r_(phi_patchT[:, kh, kw, :].rearrange(
                            "p (a b) -> p a b", b=WS)),
                        in_=phiT_pad[:, kh:kh + 16, kw:kw + 16])
            sq = ttmp.tile([CI, 324], F32, tag="sq")
            nc.scalar.activation(r_(sq),
                                 phiT_pad.rearrange("p a b -> p (a b)"),
                                 Act.Square)
            ps_n2 = ps_misc.tile([1, 324], F32, tag="m")
            nc.tensor.matmul(ps_n2, r_(ones32), r_(sq), start=True, stop=True)
            nc.scalar.copy(out=n2p, in_=ps_n2)
            n2v = n2p.rearrange("p (a b) -> p a b", b=18)
            nrm3 = nrm.rearrange("p (a b) -> p a b", b=WS)
            nc.vector.tensor_add(nrm3, n2v[:, 0:16, 0:16], n2v[:, 0:16, 1:17])
            for kh in range(3):
                for kw in range(3):
                    if kh == 0 and kw < 2:
                        continue
                    nc.vector.tensor_add(
                        nrm3, nrm3, n2v[:, kh:kh + 16, kw:kw + 16])
            nc.scalar.sqrt(nrm, nrm)
            nc.vector.tensor_scalar_max(nrm, nrm, 1e-6)
            nc.vector.reciprocal(nrm, nrm)
            nc.vector.tensor_scalar_mul(nrm, nrm, 10.0)
            nc.sync.dma_start(out=sbounce, in_=nrm)
            nc.sync.dma_start(
                out=s10T, in_=sbounce.rearrange("(t p) -> p t", p=128))

        # ---- stage 2: fused scores/softmax/deconv pipeline ----
        with ExitStack() as st2:
            e2 = st2.enter_context
            kgp = e2(tc.tile_pool(name="kgp", bufs=1))
            rbp = e2(tc.tile_pool(name="rbp", bufs=3))
            schp = e2(tc.tile_pool(name="schp", bufs=2))
            trp = e2(tc.tile_pool(name="trp", bufs=3))

            # gather the 18 dynamic-filter tiles from g_pad
            # kg[q,qw,kb][(i,j), r, (rw c)] = g_pad[4(i+q)+r, 4(j+qw)+rw, c]
            gp_r = g_pad.rearrange(
                "(hq hr) (wq wr) c -> hq hr wq (wr c)", hr=4, wr=4)
            kg = {}
            for q in range(3):
                for qw in range(3):
                    for kb in range(2):
                        t_ = kgp.tile([128, 4, 256], F32,
                                      tag=f"kg{q}{qw}{kb}",
                                      name=f"kg{q}{qw}{kb}")
                        gsrc = gp_r[kb * 8 + q: kb * 8 + q + 8,
                                    :, qw: qw + 16, :].transpose([0, 2, 1, 3])
                        for i in range(8):
                            nc.sync.dma_start(
                                out=r_(t_[i * 16:(i + 1) * 16]),
                                in_=r_(gsrc[i]))
                        kg[(q, qw, kb)] = t_.rearrange("p r x -> p (r x)")

            yr = y_h.ap().rearrange(
                "(M r) (Mw w) c -> M Mw r w c", r=4, w=4)
            rb = {}
            pending = [None]

            def drain(pend):
                tr_in, pc, u = pend
                ps_t2 = ps_tr.tile([128, 512], F32, tag="tt",
                                   name=f"ps_tr{pc}_{u}")
                for k in range(4):
                    nc.tensor.transpose(
                        ps_t2[:, k * 128:(k + 1) * 128],
                        tr_in[:, k * 128:(k + 1) * 128], ident)
                st_ = staging.tile([128, 512], F32, tag="stg",
                                   name=f"st{pc}_{u}")
                nc.scalar.copy(out=st_, in_=ps_t2)
                st3 = st_.rearrange("p (k rw c) -> p k rw c", k=4, rw=2)
                rr = u // 2
                w0 = 2 * (u % 2)
                for k in range(4):
                    for p1 in range(2):
                        nc.sync.dma_start(
                            out=yr[pc * 8 + 2 * k + p1, :, rr, w0:w0 + 2, :],
                            in_=st3[p1 * 64:(p1 + 1) * 64, k])

            def deconv_pc(pc):
                h0 = pc * 8
                for u in range(8):
                    ps_o = ps_d.tile([128, 512], F32, tag="d",
                                     name=f"ps_o{pc}_{u}")
                    first = True
                    for q in range(3):
                        for qw in range(3):
                            for kb in range(2):
                                nc.tensor.matmul(
                                    ps_o,
                                    r_(kg[(q, qw, kb)][:, u * 128:
                                                       (u + 1) * 128]),
                                    r_(attnT[:, kb, h0 + 2 - q:h0 + 10 - q,
                                             2 - qw:66 - qw]),
                                    start=first,
                                    stop=(q == 2 and qw == 2 and kb == 1))
                                first = False
                    tr_in = trp.tile([128, 512], F32, tag="ti",
                                     name=f"ti{pc}_{u}")
                    nc.scalar.copy(out=tr_in, in_=ps_o)
                    if pending[0] is not None:
                        drain(pending[0])
                    pending[0] = (tr_in, pc, u)

            for ch in range(8):
                h0 = ch * 8
                # scoresT for both n-blocks, then E = exp(s10*score)
                for kb in range(2):
                    ps_s = ps_sc.tile([128, 512], F32, tag="sc",
                                      name=f"ps_s{ch}_{kb}")
                    first = True
                    for kh in range(3):
                        for kw in range(3):
                            nc.tensor.matmul(
                                ps_s,
                                r_(phi_patchT[:, kh, kw,
                                              kb * 128:(kb + 1) * 128]),
                                r_(thetaT_pad[:, h0 + kh:h0 + kh + 8,
                                              kw:kw + 64]),
                                start=first, stop=(kh == 2 and kw == 2))
                            first = False
                    nc.scalar.activation(
                        out=r_(attnT[:, kb, 1 + h0:9 + h0, 1:65]),
                        in_=ps_s.rearrange("p (a b) -> p a b", b=64),
                        func=Act.Exp, scale=s10T[:, kb:kb + 1])
                # S = sum_n E (ones-matmul), rb = 1/S broadcast
                ps_S = ps_misc.tile([1, 512], F32, tag="m", name=f"ps_S{ch}")
                for kb in range(2):
                    nc.tensor.matmul(
                        ps_S, r_(ones128),
                        r_(attnT[:, kb, 1 + h0:9 + h0, 1:65]),
                        start=(kb == 0), stop=(kb == 1))
                sch = schp.tile([1, 512], F32, tag="sch", name=f"sch{ch}")
                nc.scalar.copy(out=sch, in_=ps_S)
                nc.vector.reciprocal(sch, sch)
                rb_t = rbp.tile([128, 512], F32, tag="rb", name=f"rb{ch}")
                nc.gpsimd.partition_broadcast(rb_t, sch)
                rb3 = rb_t.rearrange("p (a b) -> p a b", b=64)
                for kb in range(2):
                    nc.vector.tensor_mul(
                        r_(attnT[:, kb, 1 + h0:9 + h0, 1:65]),
                        attnT[:, kb, 1 + h0:9 + h0, 1:65], rb3)
                rb[ch] = rb_t
                if kdebug:
                    nc.sync.dma_start(out=dbgS_h.ap()[ch], in_=sch)
                if ch >= 1:
                    deconv_pc(ch - 1)
            deconv_pc(7)
            drain(pending[0])
            if kdebug:
                nc.sync.dma_start(out=dbgE_h.ap(), in_=attnT)

    nc.finalize()
    return nc


def kernel(**inputs):
    from concourse.bass_utils import run_bass_kernel_spmd

    if "nc" not in _CACHE:
        _CACHE["nc"] = _build_nc()
    nc = _CACHE["nc"]

    arrs = {k: np.ascontiguousarray(np.asarray(v, dtype=np.float32))
            for k, v in inputs.items()}
    x = arrs.pop("x")
    in_maps = [dict(arrs, x=x[b]) for b in range(B)]
    res = run_bass_kernel_spmd(nc, in_maps, core_ids=list(range(B)))
    return np.stack([res.results[b]["y"] for b in range(B)])
